# revision 18
# baseline (speedup 1.0000x reference)
"""Trainium2 Bass kernel for a species-routed MoE readout layer.

Math (see problem reference): per atom x [512]:
  u = silu(emb[species]); scores = softmax(u @ Wr.T)  -> top-2 sparse gates
  out = sum_e gate_e * (W2_e @ silu(W1_e @ x + b1_e) + b2_e)
      + sum_s (W2_s @ silu(W1_s @ x + b1_s) + b2_s)          # 2 shared experts

The router depends only on species_idx (64 species), so the per-atom top-2
gates collapse to a host-computed 64x6 lookup table. Atoms are grouped by
their top-2 expert pair and each group is split evenly across the 8 cores so
the single SPMD program sees the same tile->active-expert pattern on every
core; interior tiles then only compute 2 routed + 2 shared expert MLPs
instead of all 8.

Precision split (the router logits are ~N(0, 0.05), so top-2 gate sums are
only ~0.35 and the routed branch carries ~3% of output variance while the
always-on shared branch carries the rest): the two routed experts run
entirely in fp8(e4m3) double-pumped matmuls (2x PE throughput) while the
shared branch stays bf16. All scale factors are powers of two (exact):
  x8       = e4m3(x);  W1r8 = e4m3(64*W1r);  W2r8 = e4m3(64*W2r)
  h        = silu(psum/64 + b1)                       (ScalarE, scale op)
  hp8      = e4m3((h + alpha_e) * (16*gate))          (DVE STT, fp8 out;
             alpha_e = lstsq(W2_e, b2_e) folds the gated b2 into the gate)
  routed L2 accumulates at 16*64 = 1024x; the shared branch accumulates into
  the SAME psum bank with W2s pre-scaled by 1024 (bf16 exponent eats it);
  final copy = psum/1024 + sum_s b2_s on DVE.
The two shared experts are merged into one 1024-hidden expert. Per <=512
atom tile the PE then runs 32 bf16 + 16 fp8dp matmuls for L1 and
16 bf16 + 8 fp8dp for L2 (~0.75x the all-bf16 cycle count).
A short PE spin + dummy activation at kernel start warms the HAM clock
gate (cold PE runs at 1.2 instead of 2.4 GHz) and the ACT table while the
first DMAs are in flight; weight tiles stream in first-use order behind
tile 0's inputs.
"""

import numpy as np
import ml_dtypes

import concourse.bass as bass
import concourse.mybir as mybir
from concourse import bacc, tile
from concourse.bass_utils import run_bass_kernel_spmd

BF16 = mybir.dt.bfloat16
FP8 = mybir.dt.float8e4
F32 = mybir.dt.float32
BF16_NP = ml_dtypes.bfloat16
FP8_NP = ml_dtypes.float8_e4m3fn

N_CORES = 8
N_ATOMS = 100000
IN_F = 512
HID = 512
HID2 = 2 * HID  # merged shared expert hidden
OUT_F = 256
N_ROUTED = 6
N_SHARED = 2
TOPK = 2
TILE_N = 512  # atoms per tile = one PSUM bank = max matmul moving dim
KC = IN_F // 128    # 4 contraction chunks for L1
MC = HID // 128     # 4 hid chunks (routed)
MC2 = HID2 // 128   # 8 hid chunks (merged shared)
OC = OUT_F // 128   # 2 out chunks

WS = 64.0    # fp8 routed weight scale (exact power of two)
GS = 16.0    # gate prescale; routed L2 lands at WS*GS = 1024x
OS = WS * GS

SPARSE = True  # compute only active routed experts per tile


def _silu(x):
    return x / (1.0 + np.exp(-x))


def _router_table(emb, W_router):
    """[64, 6] sparse top-2 gate table + per-species expert pair."""
    u = _silu(emb.astype(np.float32))
    logits = u @ W_router.astype(np.float32).T
    m = logits.max(axis=-1, keepdims=True)
    e = np.exp(logits - m)
    scores = e / e.sum(axis=-1, keepdims=True)
    order = np.argsort(-scores, axis=-1, kind="stable")
    top2 = order[:, :TOPK]
    wt = np.zeros_like(scores)
    rows = np.arange(scores.shape[0])[:, None]
    wt[rows, top2] = scores[rows, top2]
    return wt, top2


def _plan_sharding(species_idx, top2):
    """Group atoms by top-2 expert pair, split each group evenly over cores.

    Returns (idx_cores [N_CORES, NL] int64 with -1 padding, tiles) where
    tiles is a list of (n_atoms, active_routed_experts) per <=512-atom tile,
    identical for every core by construction.
    """
    n = species_idx.shape[0]
    if not SPARSE:
        assert n % N_CORES == 0
        nl = n // N_CORES
        idx_cores = np.arange(n, dtype=np.int64).reshape(N_CORES, nl)
        tiles = []
        for t0 in range(0, nl, TILE_N):
            tiles.append((min(TILE_N, nl - t0), tuple(range(N_ROUTED))))
        return idx_cores, tiles

    MIN_TILE = 64  # merge segments smaller than this into their neighbor

    pair_of_species = [tuple(sorted(top2[s])) for s in range(top2.shape[0])]
    pairs = sorted(set(pair_of_species))
    pair_id_of_species = np.array(
        [pairs.index(p) for p in pair_of_species], dtype=np.int64
    )
    atom_pair = pair_id_of_species[species_idx]

    seg_lens = []       # per-group per-core segment length
    seg_experts = []
    group_idx = []      # per-group atom index arrays
    for g, p in enumerate(pairs):
        idx_g = np.nonzero(atom_pair == g)[0]
        if idx_g.size == 0:
            continue
        L = -(-idx_g.size // N_CORES)  # ceil
        seg_lens.append(L)
        seg_experts.append(tuple(int(x) for x in p))
        group_idx.append(idx_g)

    # largest group first: deep pipeline while the clock warms, short tail
    order = np.argsort([-L for L in seg_lens], kind="stable")
    seg_lens = [seg_lens[i] for i in order]
    seg_experts = [seg_experts[i] for i in order]
    group_idx = [group_idx[i] for i in order]

    nl = sum(seg_lens)
    idx_cores = np.full((N_CORES, nl), -1, dtype=np.int64)
    off = 0
    for L, idx_g in zip(seg_lens, group_idx):
        for c in range(N_CORES):
            part = idx_g[c * L : (c + 1) * L]
            idx_cores[c, off : off + part.size] = part
        off += L

    # Variable-size tiles aligned to segment boundaries: each tile covers a
    # single expert pair (tiny segments merge into their neighbor).
    tiles = []
    pend_n, pend_e = 0, set()
    for L, p in zip(seg_lens, seg_experts):
        pend_n += L
        pend_e.update(p)
        if pend_n < MIN_TILE:
            continue
        k = -(-pend_n // TILE_N)
        q, r = divmod(pend_n, k)
        for i in range(k):
            tiles.append((q + (1 if i < r else 0), tuple(sorted(pend_e))))
        pend_n, pend_e = 0, set()
    if pend_n:
        if tiles:
            n0, e0 = tiles.pop()
            pend_n += n0
            pend_e.update(e0)
        k = -(-pend_n // TILE_N)
        q, r = divmod(pend_n, k)
        ee = tuple(sorted(pend_e))
        for i in range(k):
            tiles.append((q + (1 if i < r else 0), ee))
    assert sum(t[0] for t in tiles) == nl
    return idx_cores, tiles


def _build_program(nl, tiles):
    nc = bacc.Bacc("TRN2", target_bir_lowering=False, debug=False)

    xT_d = nc.declare_dram_parameter("xT", [IN_F, nl], BF16, isOutput=False)
    x8_d = nc.declare_dram_parameter("x8", [IN_F, nl], FP8, isOutput=False)
    w6_d = nc.declare_dram_parameter("w6", [N_ROUTED, nl], BF16, isOutput=False)
    w1r_d = nc.declare_dram_parameter("w1r", [N_ROUTED, IN_F, HID], FP8, isOutput=False)
    w2r_d = nc.declare_dram_parameter("w2r", [N_ROUTED, HID, OUT_F], FP8, isOutput=False)
    w1s_d = nc.declare_dram_parameter("w1s", [IN_F, HID2], BF16, isOutput=False)
    w2s_d = nc.declare_dram_parameter("w2s", [HID2, OUT_F], BF16, isOutput=False)
    b1r_d = nc.declare_dram_parameter("b1r", [128, N_ROUTED * MC], F32, isOutput=False)
    b1s_d = nc.declare_dram_parameter("b1s", [128, MC2], F32, isOutput=False)
    # alpha[e] solves W2_e @ alpha_e = b2_e (host lstsq), so the gated b2
    # rides the gate multiply: W2_e @ (w*(h+alpha)) = w*(W2_e h) + w*b2_e
    alpha_d = nc.declare_dram_parameter(
        "alpha", [128, N_ROUTED * MC], F32, isOutput=False
    )
    outT_d = nc.declare_dram_parameter("outT", [OUT_F, nl], F32, isOutput=True)

    with tile.TileContext(nc) as tc:
        with (
            tc.tile_pool(name="consts", bufs=1) as consts,
            tc.tile_pool(name="xp", bufs=3) as xp,
            tc.tile_pool(name="x8p", bufs=3) as x8p,
            tc.tile_pool(name="w6p", bufs=2) as w6p,
            tc.tile_pool(name="wbcp", bufs=6) as wbcp,
            tc.tile_pool(name="hps", bufs=4, space="PSUM") as hpsp,
            tc.tile_pool(name="hp", bufs=6) as hp_pool,
            tc.tile_pool(name="hs", bufs=10) as hs_pool,
            tc.tile_pool(name="hpp", bufs=6) as hpp_pool,
            tc.tile_pool(name="ops", bufs=4, space="PSUM") as outps_pool,
            tc.tile_pool(name="osb", bufs=5) as osb_pool,
        ):
            # ---- constants / biases ----
            b1r_sb = consts.tile([128, N_ROUTED * MC], F32, name="b1r_sb")
            b1s_sb = consts.tile([128, MC2], F32, name="b1s_sb")
            alpha_sb = consts.tile([128, N_ROUTED * MC], F32, name="alpha_sb")
            ones_row = consts.tile([1, TILE_N], BF16, name="ones_row")

            # memset beats DMA here: the DMA subsystem takes ~9us to move
            # its first payload byte, Vector memsets are ready by ~3us
            ones_st = consts.tile([1, 128], BF16, name="ones_st")
            nc.vector.memset(ones_st[:], 1.0)
            nc.vector.memset(ones_row[:], 1.0)

            # Expert weight tiles stream in first-use order so tile 0's
            # matmuls can start as soon as its own experts have landed.
            eorder = []
            for _, routed in tiles:
                for e in routed:
                    if e not in eorder:
                        eorder.append(e)
                if len(eorder) == N_ROUTED:
                    break
            for e in range(N_ROUTED):
                if e not in eorder:
                    eorder.append(e)

            w1r_v = w1r_d.rearrange("e (k p) h -> e p k h", p=128)
            w2r_v = w2r_d.rearrange("e (m p) o -> e p m o", p=128)
            w1s_v = w1s_d.rearrange("(k p) h -> p k h", p=128)
            w2s_v = w2s_d.rearrange("(m p) o -> p m o", p=128)
            xT_v = xT_d.rearrange("(k p) a -> p k a", p=128)
            x8_v = x8_d.rearrange("(k p) a -> p k a", p=128)
            outT_v = outT_d.rearrange("(c p) a -> p c a", p=128)

            w1s_sb = consts.tile([128, KC, HID2], BF16, name="w1s_sb")
            w2s_sb = consts.tile([128, MC2, OUT_F], BF16, name="w2s_sb")
            w1r_sb = {}
            w2r_sb = {}

            def load_expert_weights(e):
                w1r_sb[e] = consts.tile([128, KC, HID], FP8, name=f"w1e{e}")
                w2r_sb[e] = consts.tile([128, MC, OUT_F], FP8, name=f"w2e{e}")
                nc.sync.dma_start(w1r_sb[e][:], w1r_v[e])
                nc.sync.dma_start(w2r_sb[e][:], w2r_v[e])

            # Tile 0 computes routed-first, so its critical DMA path is just
            # x8 + the pair's fp8 W1 (~0.75MB); everything else streams
            # behind it in first-use order.
            n0 = tiles[0][0]
            tile0_x8 = x8p.tile([128, KC, TILE_N], FP8, name="x8_sb", tag="x8")
            nc.sync.dma_start(tile0_x8[:, :, :n0], x8_v[:, :, :n0])
            tile0_w6 = w6p.tile([1, N_ROUTED, TILE_N], BF16, name="w6row", tag="w6r")
            nc.sync.dma_start(tile0_w6[0:1, :, :n0], w6_d[:, :n0])
            for e in eorder[:2]:
                w1r_sb[e] = consts.tile([128, KC, HID], FP8, name=f"w1e{e}")
                nc.sync.dma_start(w1r_sb[e][:], w1r_v[e])
            for e in eorder[:2]:
                w2r_sb[e] = consts.tile([128, MC, OUT_F], FP8, name=f"w2e{e}")
                nc.sync.dma_start(w2r_sb[e][:], w2r_v[e])
            nc.sync.dma_start(b1r_sb[:], b1r_d[:])
            nc.sync.dma_start(alpha_sb[:], alpha_d[:])
            tile0_x = xp.tile([128, KC, TILE_N], BF16, name="x_sb", tag="x")
            nc.sync.dma_start(tile0_x[:, :, :n0], xT_v[:, :, :n0])
            nc.sync.dma_start(w1s_sb[:], w1s_v)
            nc.sync.dma_start(b1s_sb[:], b1s_d[:])
            nc.sync.dma_start(w2s_sb[:], w2s_v)

            # Warm the PE HAM clock gate (cold = 1.2 GHz until ~3.4us of
            # sustained activity) and the ScalarE activation table while the
            # first input DMAs are in flight.
            warm_sb = consts.tile([128, 1], F32, name="warm_sb")
            for _ in range(14):
                warm_ps = hpsp.tile([128, TILE_N], F32, name="warm_ps", tag="hps")
                nc.tensor.matmul(
                    warm_ps[:, :], ones_st[0:1, :], ones_row[0:1, :],
                    start=True, stop=True,
                )
            nc.scalar.activation(
                warm_sb[:, :], b1s_sb[:, 0:1],
                mybir.ActivationFunctionType.Silu,
            )

            # ---- main loop over atom tiles ----
            a0 = 0
            for t, (n, routed) in enumerate(tiles):
                if t == 0:
                    x_sb, x8_sb, w6row = tile0_x, tile0_x8, tile0_w6
                else:
                    x_sb = xp.tile([128, KC, TILE_N], BF16, name="x_sb", tag="x")
                    nc.sync.dma_start(x_sb[:, :, :n], xT_v[:, :, a0 : a0 + n])
                    x8_sb = x8p.tile([128, KC, TILE_N], FP8, name="x8_sb", tag="x8")
                    nc.sync.dma_start(x8_sb[:, :, :n], x8_v[:, :, a0 : a0 + n])
                    # gate rows on partition 0 (prescaled by GS on host)
                    w6row = w6p.tile(
                        [1, N_ROUTED, TILE_N], BF16, name="w6row", tag="w6r"
                    )
                    nc.sync.dma_start(w6row[0:1, :, :n], w6_d[:, a0 : a0 + n])

                if t == 0:
                    # stream the remaining experts' weights behind tile 0's
                    # inputs; tile 0's compute covers the transfer time
                    for e in eorder[2:]:
                        load_expert_weights(e)

                # per-atom gates broadcast across 128 partitions (GPSIMD,
                # keeps PE free)
                wsb = {}
                for e in routed:
                    wsb_e = wbcp.tile([128, TILE_N], BF16, name="wsb", tag="wbc")
                    nc.gpsimd.partition_broadcast(
                        wsb_e[:, :n], w6row[0:1, e, :n]
                    )
                    wsb[e] = wsb_e

                # output accumulators (shared + routed both land here;
                # shared W2 is prescaled by OS so the scales match)
                outps = [
                    outps_pool.tile([128, TILE_N], F32, name="ops", tag="ops")
                    for _ in range(OC)
                ]

                def emit_routed(ps_start, ps_stop):
                    # routed experts: fp8 double-pumped L1 + L2
                    for ei, e in enumerate(routed):
                        hp8 = hpp_pool.tile(
                            [128, MC, TILE_N], FP8, name="hp8", tag="hp8"
                        )
                        for m in range(MC):
                            hps = hpsp.tile(
                                [128, TILE_N], F32, name="hps", tag="hps"
                            )
                            for k in range(0, KC, 2):
                                nc.tensor.matmul(
                                    hps[:, :n],
                                    w1r_sb[e][:, k : k + 2, m * 128 : (m + 1) * 128],
                                    x8_sb[:, k : k + 2, :n],
                                    start=(k == 0),
                                    stop=(k == KC - 2),
                                    perf_mode=mybir.MatmulPerfMode.DoubleRow,
                                )
                            h_sb = hp_pool.tile(
                                [128, TILE_N], BF16, name="h_sb", tag="h"
                            )
                            ac = e * MC + m
                            nc.scalar.activation(
                                h_sb[:, :n], hps[:, :n],
                                mybir.ActivationFunctionType.Silu,
                                bias=b1r_sb[:, ac : ac + 1],
                                scale=1.0 / WS,
                            )
                            nc.vector.scalar_tensor_tensor(
                                hp8[:, m, :n],
                                h_sb[:, :n],
                                alpha_sb[:, ac : ac + 1],
                                wsb[e][:, :n],
                                mybir.AluOpType.add,
                                mybir.AluOpType.mult,
                            )
                        last_e = ei == len(routed) - 1
                        for c in range(OC):
                            for mp in range(0, MC, 2):
                                nc.tensor.matmul(
                                    outps[c][:, :n],
                                    w2r_sb[e][:, mp : mp + 2, c * 128 : (c + 1) * 128],
                                    hp8[:, mp : mp + 2, :n],
                                    start=(ps_start and ei == 0 and mp == 0),
                                    stop=(ps_stop and last_e and mp == MC - 2),
                                    perf_mode=mybir.MatmulPerfMode.DoubleRow,
                                )

                def emit_shared(ps_start, ps_stop):
                    # merged 1024-hidden shared expert, bf16
                    for m in range(MC2):
                        hps = hpsp.tile([128, TILE_N], F32, name="hps", tag="hps")
                        for k in range(KC):
                            nc.tensor.matmul(
                                hps[:, :n],
                                w1s_sb[:, k, m * 128 : (m + 1) * 128],
                                x_sb[:, k, :n],
                                start=(k == 0),
                                stop=(k == KC - 1),
                            )
                        h_sb = hp_pool.tile([128, TILE_N], BF16, name="h_sb", tag="h")
                        nc.scalar.activation(
                            h_sb[:, :n], hps[:, :n],
                            mybir.ActivationFunctionType.Silu,
                            bias=b1s_sb[:, m : m + 1],
                        )
                        for c in range(OC):
                            nc.tensor.matmul(
                                outps[c][:, :n],
                                w2s_sb[:, m, c * 128 : (c + 1) * 128],
                                h_sb[:, :n],
                                start=(ps_start and m == 0),
                                stop=(ps_stop and m == MC2 - 1),
                            )

                def routed_l1_chunk(e, m, hp8):
                    hps = hpsp.tile([128, TILE_N], F32, name="hps", tag="hps")
                    for k in range(0, KC, 2):
                        nc.tensor.matmul(
                            hps[:, :n],
                            w1r_sb[e][:, k : k + 2, m * 128 : (m + 1) * 128],
                            x8_sb[:, k : k + 2, :n],
                            start=(k == 0),
                            stop=(k == KC - 2),
                            perf_mode=mybir.MatmulPerfMode.DoubleRow,
                        )
                    h_sb = hp_pool.tile([128, TILE_N], BF16, name="h_sb", tag="h")
                    ac = e * MC + m
                    nc.scalar.activation(
                        h_sb[:, :n], hps[:, :n],
                        mybir.ActivationFunctionType.Silu,
                        bias=b1r_sb[:, ac : ac + 1],
                        scale=1.0 / WS,
                    )
                    nc.vector.scalar_tensor_tensor(
                        hp8[:, m, :n],
                        h_sb[:, :n],
                        alpha_sb[:, ac : ac + 1],
                        wsb[e][:, :n],
                        mybir.AluOpType.add,
                        mybir.AluOpType.mult,
                    )

                def shared_l1_chunk(m):
                    hps = hpsp.tile([128, TILE_N], F32, name="hps", tag="hps")
                    for k in range(KC):
                        nc.tensor.matmul(
                            hps[:, :n],
                            w1s_sb[:, k, m * 128 : (m + 1) * 128],
                            x_sb[:, k, :n],
                            start=(k == 0),
                            stop=(k == KC - 1),
                        )
                    h_sb = hs_pool.tile([128, TILE_N], BF16, name="hs_sb", tag="hs")
                    nc.scalar.activation(
                        h_sb[:, :n], hps[:, :n],
                        mybir.ActivationFunctionType.Silu,
                        bias=b1s_sb[:, m : m + 1],
                    )
                    return h_sb

                if t == 0:
                    # tile 0 leads with routed: its DMA critical path is just
                    # x8 + the pair's fp8 W1 (~0.75MB), shared weights stream
                    # behind it
                    emit_routed(True, False)
                    emit_shared(False, True)
                else:
                    # Interleave shared (4 bf16 mm -> 1 silu) with routed
                    # (2 fp8dp mm -> 1 silu + 1 STT) L1 chunks so ScalarE's
                    # silu demand tracks the PE's psum production instead of
                    # bursting 2x past it in an all-routed phase; both L2
                    # phases then run with their inputs already drained.
                    hp8s = {
                        e: hpp_pool.tile(
                            [128, MC, TILE_N], FP8, name="hp8", tag="hp8"
                        )
                        for e in routed
                    }
                    rchunks = [(e, m) for e in routed for m in range(MC)]
                    hs = []
                    for m in range(MC2):
                        hs.append(shared_l1_chunk(m))
                        if m < len(rchunks):
                            e, rm = rchunks[m]
                            routed_l1_chunk(e, rm, hp8s[e])
                    for e, rm in rchunks[MC2:]:
                        routed_l1_chunk(e, rm, hp8s[e])
                    # shared L2 first: routed L2's hp8 STT chain drains behind
                    for m in range(MC2):
                        for c in range(OC):
                            nc.tensor.matmul(
                                outps[c][:, :n],
                                w2s_sb[:, m, c * 128 : (c + 1) * 128],
                                hs[m][:, :n],
                                start=(m == 0),
                                stop=False,
                            )
                    for ei, e in enumerate(routed):
                        last_e = ei == len(routed) - 1
                        for c in range(OC):
                            for mp in range(0, MC, 2):
                                nc.tensor.matmul(
                                    outps[c][:, :n],
                                    w2r_sb[e][:, mp : mp + 2, c * 128 : (c + 1) * 128],
                                    hp8s[e][:, mp : mp + 2, :n],
                                    start=False,
                                    stop=(last_e and mp == MC - 2),
                                    perf_mode=mybir.MatmulPerfMode.DoubleRow,
                                )

                # psum -> sbuf descale, split across ScalarE and DVE so the
                # two banks drain in parallel (sum_s b2_s is a constant per
                # output channel; the host adds it during the final gather)
                osb = osb_pool.tile([128, OC, TILE_N], F32, name="osb", tag="osb")
                nc.scalar.activation(
                    osb[:, 0, :n], outps[0][:, :n],
                    mybir.ActivationFunctionType.Copy,
                    bias=0.0, scale=1.0 / OS,
                )
                nc.vector.tensor_scalar_mul(
                    osb[:, 1, :n], outps[1][:, :n], 1.0 / OS
                )
                nc.sync.dma_start(
                    outT_v[:, :, a0 : a0 + n], osb[:, :, :n]
                )
                a0 += n

    nc.compile()
    return nc


def _alpha_pack(rW2, rb2):
    """alpha_e = min-norm solution of W2_e @ alpha = b2_e, packed per-chunk."""
    alphas = []
    for e in range(N_ROUTED):
        a, *_ = np.linalg.lstsq(rW2[e].astype(np.float64), rb2[e].astype(np.float64))
        alphas.append(a)
    al = np.stack(alphas).astype(np.float32)  # [6, HID]
    return np.ascontiguousarray(
        al.reshape(N_ROUTED, MC, 128).transpose(2, 0, 1).reshape(128, N_ROUTED * MC)
    )


def _prep_host(inputs):
    feats = np.asarray(inputs["features"], dtype=np.float32)
    species = np.asarray(inputs["species_idx"]).astype(np.int64)
    emb = np.asarray(inputs["emb"], dtype=np.float32)
    Wr = np.asarray(inputs["W_router"], dtype=np.float32)
    rW1 = np.asarray(inputs["rW1"], dtype=np.float32)
    rb1 = np.asarray(inputs["rb1"], dtype=np.float32)
    rW2 = np.asarray(inputs["rW2"], dtype=np.float32)
    rb2 = np.asarray(inputs["rb2"], dtype=np.float32)
    sW1 = np.asarray(inputs["sW1"], dtype=np.float32)
    sb1 = np.asarray(inputs["sb1"], dtype=np.float32)
    sW2 = np.asarray(inputs["sW2"], dtype=np.float32)
    sb2 = np.asarray(inputs["sb2"], dtype=np.float32)

    wt_table, top2 = _router_table(emb, Wr)
    idx_cores, tiles = _plan_sharding(species, top2)
    nl = idx_cores.shape[1]
    w_atoms = wt_table[species]  # [n, 6] f32

    # merged shared expert: [1024, 512] W1, [256, 1024] W2
    W1s = np.concatenate([sW1[s] for s in range(N_SHARED)], axis=0)
    W2s = np.concatenate([sW2[s] for s in range(N_SHARED)], axis=1)
    b1s = np.concatenate([sb1[s] for s in range(N_SHARED)], axis=0)  # [1024]

    shared = {
        "w1r": np.ascontiguousarray(
            (rW1 * WS).transpose(0, 2, 1)).astype(FP8_NP),
        "w2r": np.ascontiguousarray(
            (rW2 * WS).transpose(0, 2, 1)).astype(FP8_NP),
        "w1s": np.ascontiguousarray(W1s.T).astype(BF16_NP),
        "w2s": np.ascontiguousarray((W2s * OS).T).astype(BF16_NP),
        "b1r": np.ascontiguousarray(
            np.concatenate([rb1], axis=0)
            .reshape(N_ROUTED, MC, 128).transpose(2, 0, 1)
            .reshape(128, N_ROUTED * MC)
        ),
        "b1s": np.ascontiguousarray(
            b1s.reshape(MC2, 128).T
        ),
        "alpha": _alpha_pack(rW2, rb2),
    }

    in_maps = []
    for c in range(N_CORES):
        idx = idx_cores[c]
        valid = idx >= 0
        iv = idx[valid]
        xT = np.zeros((IN_F, nl), dtype=BF16_NP)
        xT[:, valid] = np.ascontiguousarray(feats[iv].T).astype(BF16_NP)
        x8 = np.zeros((IN_F, nl), dtype=FP8_NP)
        x8[:, valid] = np.ascontiguousarray(feats[iv].T).astype(FP8_NP)
        w6 = np.zeros((N_ROUTED, nl), dtype=BF16_NP)
        w6[:, valid] = np.ascontiguousarray(
            (w_atoms[iv] * GS).T).astype(BF16_NP)
        in_maps.append({"xT": xT, "x8": x8, "w6": w6, **shared})
    return in_maps, idx_cores, tiles, nl, feats.shape[0]


_PROGRAM_CACHE = {}


def _get_program(nl, tiles):
    key = (nl, tuple(tiles))
    if key not in _PROGRAM_CACHE:
        _PROGRAM_CACHE[key] = _build_program(nl, tiles)
    return _PROGRAM_CACHE[key]


# Set TRACE=True (e.g. from a test harness) to capture a neuron-profile trace;
# the full BassKernelResults of the last run is kept in LAST_RESULTS.
TRACE = False
LAST_RESULTS = None


def kernel(**inputs):
    global LAST_RESULTS
    in_maps, idx_cores, tiles, nl, n_atoms = _prep_host(inputs)
    nc = _get_program(nl, tiles)
    res = run_bass_kernel_spmd(nc, in_maps, list(range(N_CORES)), trace=TRACE)
    LAST_RESULTS = res
    out = np.zeros((n_atoms, OUT_F), dtype=np.float32)
    for c in range(N_CORES):
        idx = idx_cores[c]
        valid = idx >= 0
        outT = res.results[c]["outT"]  # [OUT_F, nl] f32
        out[idx[valid]] = outT[:, valid].T
    out += np.asarray(inputs["sb2"], dtype=np.float32).sum(axis=0)
    return out
